# revision 4
# baseline (speedup 1.0000x reference)
"""Trainium2 Bass kernel for nn_Attention (B=4,T=2048,C=512,H=8 causal RoPE attention).

Sharding: 8 cores = 4 batches x 2 head-groups. Core c handles batch c//2 and
heads [4*(c%2), 4*(c%2)+4). Each core computes its proj partial y_part[T, C]
in bf16; the host sums the two partials per batch (f32) and adds bp.

v2 design (ACT-exp-bound pipeline):
  - qT = Wq_loc @ x^T; pair-swapped copy via DVE stream_shuffle (no swap
    weights); RoPE muls split DVE/GPSIMD.
  - Scores transposed S^T[kt, qt] per head, TWO heads per psum group
    [128, 1024]: h0 cols 0:512 (bank a), h1 cols 512:1024 (bank b) via
    row-tiled concurrent matmuls (K=64 at tile rows 0/64).
  - Causal staircase: diagonal kt-tile r computes only qt cols >= 128r.
    One strided exp [128, 2, 512-128r] covers both heads; triangle mask
    multiply (128-wide) on GPSIMD.
  - PV: (V|1)-stationary M=65 matmuls accumulate out^T + denominator row.
    Normalize via ones-broadcast matmul + reciprocal + mul.
  - Aggressive overlap: v-proj and head-pair-1 projections interleaved into
    the ACT-bound attention phase; output proj interleaved with final
    normalizes; bf16 output DMA.
"""

import sys

for _p in ("/opt/trn_rl_repo",):
    if _p not in sys.path:
        sys.path.insert(0, _p)

from contextlib import ExitStack

import ml_dtypes
import numpy as np

import concourse.bass as bass
import concourse.tile as tile
from concourse import bacc
from concourse import mybir
from concourse.bass_utils import run_bass_kernel_spmd


def _ensure_ntff_hook():
    """Provide antenv.axon_hooks (missing in this image) so trace=True works."""
    try:
        import antenv.axon_hooks  # noqa: F401

        return
    except ImportError:
        pass
    import contextlib
    import ctypes
    import types

    import antenv

    mod = types.ModuleType("antenv.axon_hooks")
    holder = {}
    mod.set_axon_ntff_profile_hook = lambda h: holder.__setitem__("h", h)
    mod.get_axon_ntff_profile_hook = lambda: holder.get("h")
    antenv.axon_hooks = mod
    sys.modules["antenv.axon_hooks"] = mod

    so_path = "/opt/axon/libaxon_pjrt.so"
    try:
        lib = ctypes.CDLL(so_path)
    except OSError:
        return
    if not hasattr(lib, "axon_start_nrt_profile"):
        return
    lib.axon_start_nrt_profile.argtypes = [
        ctypes.POINTER(ctypes.c_int64),
        ctypes.c_size_t,
    ]
    lib.axon_start_nrt_profile.restype = ctypes.c_int64
    lib.axon_stop_nrt_profile.argtypes = [ctypes.c_char_p]
    lib.axon_stop_nrt_profile.restype = ctypes.c_int64

    @contextlib.contextmanager
    def _hook(output_dir, device_ids):
        import jax

        jax.devices()
        if device_ids:
            ids = (ctypes.c_int64 * len(device_ids))(*device_ids)
            rc = lib.axon_start_nrt_profile(ids, len(device_ids))
        else:
            rc = lib.axon_start_nrt_profile(None, 0)
        if rc != 0:
            raise RuntimeError(f"axon_start_nrt_profile rc={rc}")
        try:
            yield
        finally:
            n = lib.axon_stop_nrt_profile(str(output_dir).encode())
            print(f"profile: {n} file(s) written to {output_dir}", file=sys.stderr)

    mod.set_axon_ntff_profile_hook(_hook)


BF16 = mybir.dt.bfloat16
F32 = mybir.dt.float32
NPBF = ml_dtypes.bfloat16

B, C, H, D = 4, 512, 8, 64
HPC = 4              # heads per core
CL = HPC * D         # 256 local channels
NCORES = 8
THETA = 10000.0
QC = 512             # q-chunk width
ACT_EXP = mybir.ActivationFunctionType.Exp

SWAP_MASK = [i ^ 1 for i in range(32)]


def build_nc(T: int) -> bass.Bass:
    PT = T // 128
    NJ = T // QC
    nc = bacc.Bacc()

    xT = nc.declare_dram_parameter("xT", [C, T], BF16, isOutput=False)
    wq = nc.declare_dram_parameter("wq", [C, CL], BF16, isOutput=False)
    wk = nc.declare_dram_parameter("wk", [C, CL], BF16, isOutput=False)
    wv = nc.declare_dram_parameter("wv", [C, CL], BF16, isOutput=False)
    wp = nc.declare_dram_parameter("wp", [CL, C], BF16, isOutput=False)
    cosb = nc.declare_dram_parameter("cosb", [CL, T], BF16, isOutput=False)
    sinb = nc.declare_dram_parameter("sinb", [CL, T], BF16, isOutput=False)
    msk = nc.declare_dram_parameter("msk", [128, 256], BF16, isOutput=False)
    y = nc.declare_dram_parameter("y", [T, C], BF16, isOutput=True)

    with nc.allow_low_precision(
        reason="bf16 compute by design; f32 PSUM accumulation everywhere"
    ), tile.TileContext(nc) as tc, ExitStack() as ctx:
        pers = ctx.enter_context(tc.tile_pool(name="pers", bufs=1))
        work = ctx.enter_context(tc.tile_pool(name="work", bufs=4))
        pexp = ctx.enter_context(tc.tile_pool(name="pexp", bufs=6))
        psc = ctx.enter_context(tc.tile_pool(name="psc", bufs=2, space="PSUM"))
        b1 = ctx.enter_context(tc.tile_pool(name="b1", bufs=3, space="PSUM"))
        bcp = ctx.enter_context(tc.tile_pool(name="bcp", bufs=1, space="PSUM"))

        # ---------------- persistent SBUF: inputs ----------------
        xT_sb = [pers.tile([128, T], BF16, name=f"xT{i}", tag=f"xT{i}") for i in range(4)]

        def wtiles(name):
            return [
                pers.tile([128, CL], BF16, name=f"{name}{i}", tag=f"{name}{i}")
                for i in range(4)
            ]

        wq_sb, wk_sb, wv_sb = wtiles("wq"), wtiles("wk"), wtiles("wv")
        for i in range(4):
            nc.sync.dma_start(out=xT_sb[i][:], in_=xT[128 * i:128 * i + 128, :])
        for i in range(4):
            nc.sync.dma_start(out=wk_sb[i][:], in_=wk[128 * i:128 * i + 128, :])
            nc.sync.dma_start(out=wq_sb[i][:], in_=wq[128 * i:128 * i + 128, :])
        cos_sb = [pers.tile([128, T], BF16, name=f"cos{i}", tag=f"cos{i}") for i in range(2)]
        sin_sb = [pers.tile([128, T], BF16, name=f"sin{i}", tag=f"sin{i}") for i in range(2)]
        nc.sync.dma_start(out=cos_sb[0][:], in_=cosb[0:128, :])
        nc.sync.dma_start(out=sin_sb[0][:], in_=sinb[0:128, :])
        for i in range(4):
            nc.sync.dma_start(out=wv_sb[i][:], in_=wv[128 * i:128 * i + 128, :])
        nc.sync.dma_start(out=cos_sb[1][:], in_=cosb[128:256, :])
        nc.sync.dma_start(out=sin_sb[1][:], in_=sinb[128:256, :])
        msk_sb = pers.tile([128, 256], BF16, name="msk", tag="msk")
        nc.sync.dma_start(out=msk_sb[:], in_=msk[:, :])
        wp_sb = [pers.tile([128, C], BF16, name=f"wp{i}", tag=f"wp{i}") for i in range(2)]
        for i in range(2):
            nc.sync.dma_start(out=wp_sb[i][:], in_=wp[128 * i:128 * i + 128, :])

        # ---------------- persistent SBUF: intermediates ----------------
        qT_sb = [pers.tile([128, T], BF16, name=f"qT{i}", tag=f"qT{i}") for i in range(2)]
        kT_sb = [pers.tile([128, T], BF16, name=f"kT{i}", tag=f"kT{i}") for i in range(2)]
        vx_sb = [pers.tile([128, HPC * (D + 1)], BF16, name=f"vx{i}", tag=f"vx{i}") for i in range(PT)]
        rnT_sb = [pers.tile([128, T], BF16, name=f"rn{i}", tag=f"rn{i}") for i in range(2)]
        den_sb = pers.tile([1, HPC * T], BF16, name="den", tag="den")
        ones_sb = pers.tile([1, 64], BF16, name="ones", tag="ones")
        nc.vector.memset(ones_sb[:], 1.0)
        # ones columns of vx (col 65h+64) are set once
        for tt in range(PT):
            v3 = vx_sb[tt][:, :].rearrange("p (h x) -> p h x", h=HPC)
            nc.vector.memset(v3[:, :, 64:65], 1.0)

        # ---------------- building blocks ----------------
        def proj_rope(m, which, t4):
            """Project+RoPE one [128, 512] tile of q or k for head pair m."""
            wn = wq_sb if which == "q" else wk_sb
            dst = qT_sb if which == "q" else kT_sb
            tsl = slice(QC * t4, QC * t4 + QC)
            pq = b1.tile([128, 512], F32, name="b1", tag="b1")
            for kc in range(4):
                nc.tensor.matmul(
                    pq[:],
                    lhsT=wn[kc][:, 128 * m:128 * m + 128],
                    rhs=xT_sb[kc][:, tsl],
                    start=(kc == 0),
                    stop=(kc == 3),
                )
            t1 = work.tile([128, 512], BF16, name="t1", tag="t1")
            t2s = work.tile([128, 512], BF16, name="t2s", tag="t2s")
            m1t = work.tile([128, 512], BF16, name="m1t", tag="m1t")
            nc.vector.tensor_copy(t1[:], pq[:])
            # pair-swapped copy (partition perm within 32-blocks)
            nc.vector.stream_shuffle(t2s[:], t1[:], SWAP_MASK)
            nc.vector.tensor_mul(m1t[:], t1[:], cos_sb[m][:, tsl])
            t2 = work.tile([128, 512], BF16, name="t2", tag="t2")
            nc.gpsimd.tensor_mul(t2[:], t2s[:], sin_sb[m][:, tsl])
            nc.gpsimd.tensor_add(dst[m][:, tsl], m1t[:], t2[:])

        def vproj(tt):
            """V projection for one 128-row t tile, interleaved (V|1) layout."""
            pv = b1.tile([128, 512], F32, name="b1", tag="b1")
            for kc in range(4):
                nc.tensor.matmul(
                    pv[:, 0:CL],
                    lhsT=xT_sb[kc][:, 128 * tt:128 * tt + 128],
                    rhs=wv_sb[kc][:],
                    start=(kc == 0),
                    stop=(kc == 3),
                )
            v3 = vx_sb[tt][:, :].rearrange("p (h x) -> p h x", h=HPC)
            p3 = pv[:, 0:CL].rearrange("p (h x) -> p h x", h=HPC)
            nc.vector.tensor_copy(v3[:, :, 0:64], p3[:, :, :])

        def attn_group(ph, j, it, pvp):
            """One attention group: kt tile `it`, both heads of pair ph."""
            nkt = 4 * (j + 1)
            r = it - 4 * j
            lo = 128 * r if r >= 0 else 0   # staircase column offset
            w = 512 - lo
            qsl = slice(QC * j + lo, QC * j + QC)
            sg = psc.tile([128, 1024], F32, name="sg", tag="sg")
            for sub in range(2):
                po = 64 * sub
                nc.tensor.matmul(
                    sg[:, 512 * sub + lo:512 * sub + 512],
                    lhsT=kT_sb[ph][po:po + 64, 128 * it:128 * it + 128],
                    rhs=qT_sb[ph][po:po + 64, qsl],
                    start=True,
                    stop=True,
                )
            pg = pexp.tile([128, 1024], BF16, name="pg", tag="pg")
            sg3 = sg[:, :].rearrange("p (b n) -> p b n", b=2)
            pg3 = pg[:, :].rearrange("p (b n) -> p b n", b=2)
            nc.scalar.activation(
                pg3[:, :, lo:512], sg3[:, :, lo:512], ACT_EXP, scale=0.125
            )
            if r >= 0:
                m3 = msk_sb[:, :].rearrange("p (b n) -> p b n", b=2)
                nc.gpsimd.tensor_mul(
                    pg3[:, :, lo:lo + 128], pg3[:, :, lo:lo + 128], m3[:, :, :]
                )
            for sub in range(2):
                h = 2 * ph + sub
                nc.tensor.matmul(
                    pvp[sub][0:65, lo:512],
                    lhsT=vx_sb[it][:, 65 * h:65 * h + 65],
                    rhs=pg[:, 512 * sub + lo:512 * sub + 512],
                    start=(it == 0),
                    stop=(it == nkt - 1),
                )

        def normalize(ph, j, pvp):
            """Denominator broadcast + reciprocal + scale for both subs."""
            qsl = slice(QC * j, QC * j + QC)
            bc = bcp.tile([128, 512], F32, name="bc", tag="bc")
            for sub in range(2):
                h = 2 * ph + sub
                dsl = slice(h * T + QC * j, h * T + QC * j + QC)
                nc.vector.tensor_copy(den_sb[0:1, dsl], pvp[sub][64:65, :])
                nc.tensor.matmul(
                    bc[64 * sub:64 * sub + 64, :],
                    lhsT=ones_sb[0:1, :],
                    rhs=den_sb[0:1, dsl],
                    start=True,
                    stop=True,
                    tile_position=(0, 64 * sub),
                )
            rbc = work.tile([128, 512], F32, name="rbc", tag="rbc")
            nc.vector.reciprocal_approx_fast(rbc[:], bc[:])
            for sub in range(2):
                nc.vector.tensor_mul(
                    rnT_sb[ph][64 * sub:64 * sub + 64, qsl],
                    pvp[sub][0:64, :],
                    rbc[64 * sub:64 * sub + 64, :],
                )

        def proj_out(tt):
            """Output projection for one 128-row t tile + store."""
            pp = b1.tile([128, 512], F32, name="b1", tag="b1")
            for kc in range(2):
                nc.tensor.matmul(
                    pp[:],
                    lhsT=rnT_sb[kc][:, 128 * tt:128 * tt + 128],
                    rhs=wp_sb[kc][:],
                    start=(kc == 0),
                    stop=(kc == 1),
                )
            ys = work.tile([128, 512], BF16, name="ys", tag="ys")
            nc.vector.tensor_copy(ys[:], pp[:])
            nc.sync.dma_start(out=y[128 * tt:128 * tt + 128, :], in_=ys[:])

        # ---------------- schedule ----------------
        # prefix: minimal m0 projections to start attention j=0
        proj_rope(0, "k", 0)
        proj_rope(0, "q", 0)
        for tt in range(4):
            vproj(tt)

        # m1 projections are pure fillers, interleaved into the ACT-bound
        # attention ph=0 phase (all emitted before ph=1 needs them).
        fillers = []
        for t4 in range(NJ):
            fillers.append(("k", t4))
            fillers.append(("q", t4))
        fi = 0

        def emit_fillers(n):
            nonlocal fi
            for _ in range(n):
                if fi >= len(fillers):
                    return
                which, t4 = fillers[fi]
                fi += 1
                proj_rope(1, which, t4)

        for ph in range(2):
            if ph == 1:
                emit_fillers(len(fillers))
            for j in range(NJ):
                if ph == 0 and j >= 1:
                    # projections this j depends on must be emitted first
                    proj_rope(0, "k", j)
                    proj_rope(0, "q", j)
                    for tt in range(4 * j, 4 * j + 4):
                        vproj(tt)
                nkt = 4 * (j + 1)
                pvp = [
                    b1.tile([128, 512], F32, name="b1", tag="b1")
                    for _ in range(2)
                ]
                for it in range(nkt):
                    attn_group(ph, j, it, pvp)
                    if ph == 0 and it % 3 == 2:
                        emit_fillers(1)
                normalize(ph, j, pvp)
                if ph == 1:
                    for tt in range(4 * j, 4 * j + 4):
                        proj_out(tt)

    nc.finalize()
    return nc


def prep_core_inputs(x, Wq, Wk, Wv, Wp, core, T):
    b, g = core // 2, core % 2
    sl = slice(CL * g, CL * g + CL)
    lc = np.arange(CL)
    gpair = (CL * g + lc) // 2
    invf = THETA ** (-(2.0 * gpair) / C)
    ang = np.arange(T)[None, :] * invf[:, None]
    cosb = np.cos(ang).astype(np.float32)
    sgn = np.where(lc % 2 == 0, -1.0, 1.0)
    sinb = (np.sin(ang) * sgn[:, None]).astype(np.float32)
    # triangular keep-mask (q >= p) duplicated for the two packed heads
    p = np.arange(128)[:, None]
    q = np.arange(128)[None, :]
    tri = (q >= p).astype(np.float32)
    m = np.concatenate([tri, tri], axis=1)
    return {
        "xT": np.ascontiguousarray(x[b].T).astype(NPBF),
        "wq": np.ascontiguousarray(Wq[sl, :].T).astype(NPBF),
        "wk": np.ascontiguousarray(Wk[sl, :].T).astype(NPBF),
        "wv": np.ascontiguousarray(Wv[sl, :].T).astype(NPBF),
        "wp": np.ascontiguousarray(Wp[:, sl].T).astype(NPBF),
        "cosb": cosb.astype(NPBF),
        "sinb": sinb.astype(NPBF),
        "msk": m.astype(NPBF),
    }


_NC_CACHE = {}


def _get_nc(T):
    if T not in _NC_CACHE:
        _NC_CACHE[T] = build_nc(T)
    return _NC_CACHE[T]


def kernel(x, Wq, Wk, Wv, Wp, bp, _trace=False):
    x = np.asarray(x, dtype=np.float32)
    Wq = np.asarray(Wq, dtype=np.float32)
    Wk = np.asarray(Wk, dtype=np.float32)
    Wv = np.asarray(Wv, dtype=np.float32)
    Wp = np.asarray(Wp, dtype=np.float32)
    bp = np.asarray(bp, dtype=np.float32)
    T = x.shape[1]
    nc = _get_nc(T)
    in_maps = [prep_core_inputs(x, Wq, Wk, Wv, Wp, c, T) for c in range(NCORES)]
    if _trace:
        _ensure_ntff_hook()
    res = run_bass_kernel_spmd(nc, in_maps, list(range(NCORES)), trace=_trace)
    out = np.zeros((B, T, C), np.float32)
    for b in range(B):
        out[b] = res.results[2 * b]["y"].astype(np.float32) + res.results[
            2 * b + 1
        ]["y"].astype(np.float32)
    out += bp[None, None, :]
    if _trace:
        return out, res
    return out


# revision 9
# speedup vs baseline: 1.1928x; 1.1928x over previous
"""Trainium2 Bass kernel for nn_Attention (B=4,T=2048,C=512,H=8 causal RoPE attention).

Sharding: 8 cores = 4 batches x 2 head-groups. Core c handles batch c//2 and
heads [4*(c%2), 4*(c%2)+4). Each core computes its proj partial y_part[T, C]
in bf16; the host sums the two partials per batch (f32) and adds bp.

v3 design (ACT-exp-bound pipeline, engine-balanced):
  - qT = Wq_loc @ x^T; pair-swapped copy via DVE stream_shuffle of the bf16
    cast (cast on ACT); RoPE muls on DVE (head-pair 0, latency-critical) or
    GPSIMD (head-pair 1, slack-filled).
  - Scores transposed S^T[kt, qt], TWO heads per psum group [128, 1024]:
    h0 cols 0:512, h1 cols 512:1024 via row-tiled concurrent matmuls (K=64
    at PE rows 0/64). Causal staircase: diagonal kt-tile r computes only
    qt cols >= 128r; one strided exp [128, 2, 512-128r] covers both heads;
    triangular mask multiply on DVE.
  - PV: (V|1)-stationary M=65 matmuls accumulate out^T + denominator row.
    pvp psum evacuated immediately via one [65,512] cast per sub into bf16
    staging (rawA/rawB); normalize (ones-broadcast matmul of the staged den
    row + reciprocal + mul) is deferred and overlapped with later attention.
  - v-proj / head-pair-1 projections / output proj interleaved into the
    ACT-bound attention phase; bf16 output DMA.
"""

import sys

for _p in ("/opt/trn_rl_repo",):
    if _p not in sys.path:
        sys.path.insert(0, _p)

from contextlib import ExitStack

import ml_dtypes
import numpy as np

import concourse.bass as bass
import concourse.tile as tile
from concourse import bacc
from concourse import mybir
from concourse.bass_utils import run_bass_kernel_spmd


def _ensure_ntff_hook():
    """Provide antenv.axon_hooks (missing in this image) so trace=True works."""
    try:
        import antenv.axon_hooks  # noqa: F401

        return
    except ImportError:
        pass
    import contextlib
    import ctypes
    import types

    import antenv

    mod = types.ModuleType("antenv.axon_hooks")
    holder = {}
    mod.set_axon_ntff_profile_hook = lambda h: holder.__setitem__("h", h)
    mod.get_axon_ntff_profile_hook = lambda: holder.get("h")
    antenv.axon_hooks = mod
    sys.modules["antenv.axon_hooks"] = mod

    so_path = "/opt/axon/libaxon_pjrt.so"
    try:
        lib = ctypes.CDLL(so_path)
    except OSError:
        return
    if not hasattr(lib, "axon_start_nrt_profile"):
        return
    lib.axon_start_nrt_profile.argtypes = [
        ctypes.POINTER(ctypes.c_int64),
        ctypes.c_size_t,
    ]
    lib.axon_start_nrt_profile.restype = ctypes.c_int64
    lib.axon_stop_nrt_profile.argtypes = [ctypes.c_char_p]
    lib.axon_stop_nrt_profile.restype = ctypes.c_int64

    @contextlib.contextmanager
    def _hook(output_dir, device_ids):
        import jax

        jax.devices()
        if device_ids:
            ids = (ctypes.c_int64 * len(device_ids))(*device_ids)
            rc = lib.axon_start_nrt_profile(ids, len(device_ids))
        else:
            rc = lib.axon_start_nrt_profile(None, 0)
        if rc != 0:
            raise RuntimeError(f"axon_start_nrt_profile rc={rc}")
        try:
            yield
        finally:
            n = lib.axon_stop_nrt_profile(str(output_dir).encode())
            print(f"profile: {n} file(s) written to {output_dir}", file=sys.stderr)

    mod.set_axon_ntff_profile_hook(_hook)


BF16 = mybir.dt.bfloat16
F32 = mybir.dt.float32
NPBF = ml_dtypes.bfloat16

B, C, H, D = 4, 512, 8, 64
HPC = 4              # heads per core
CL = HPC * D         # 256 local channels
NCORES = 8
THETA = 10000.0
QC = 512             # q-chunk width
ACT_EXP = mybir.ActivationFunctionType.Exp

SWAP_MASK = [i ^ 1 for i in range(32)]


def build_nc(T: int) -> bass.Bass:
    PT = T // 128
    NJ = T // QC
    nc = bacc.Bacc()

    xT = nc.declare_dram_parameter("xT", [C, T], BF16, isOutput=False)
    wq = nc.declare_dram_parameter("wq", [C, CL], BF16, isOutput=False)
    wk = nc.declare_dram_parameter("wk", [C, CL], BF16, isOutput=False)
    wv = nc.declare_dram_parameter("wv", [C, CL], BF16, isOutput=False)
    wp = nc.declare_dram_parameter("wp", [CL, C], BF16, isOutput=False)
    cosb = nc.declare_dram_parameter("cosb", [CL, T], BF16, isOutput=False)
    sinb = nc.declare_dram_parameter("sinb", [CL, T], BF16, isOutput=False)
    msk = nc.declare_dram_parameter("msk", [128, 256], BF16, isOutput=False)
    y = nc.declare_dram_parameter("y", [T, C], BF16, isOutput=True)

    with nc.allow_low_precision(
        reason="bf16 compute by design; f32 PSUM accumulation everywhere"
    ), tile.TileContext(nc) as tc, ExitStack() as ctx:
        pers = ctx.enter_context(tc.tile_pool(name="pers", bufs=1))
        work = ctx.enter_context(tc.tile_pool(name="work", bufs=4))
        pexp = ctx.enter_context(tc.tile_pool(name="pexp", bufs=6))
        psc = ctx.enter_context(tc.tile_pool(name="psc", bufs=2, space="PSUM"))
        b1 = ctx.enter_context(tc.tile_pool(name="b1", bufs=3, space="PSUM"))
        bcp = ctx.enter_context(tc.tile_pool(name="bcp", bufs=1, space="PSUM"))

        # ---------------- persistent SBUF: inputs ----------------
        xT_sb = [pers.tile([128, T], BF16, name=f"xT{i}", tag=f"xT{i}") for i in range(4)]

        def wtiles(name):
            return [
                pers.tile([128, CL], BF16, name=f"{name}{i}", tag=f"{name}{i}")
                for i in range(4)
            ]

        wq_sb, wk_sb, wv_sb = wtiles("wq"), wtiles("wk"), wtiles("wv")
        # first q-chunk of x lands first so the first projections start early
        for i in range(4):
            nc.sync.dma_start(out=xT_sb[i][:, 0:QC], in_=xT[128 * i:128 * i + 128, 0:QC])
        for i in range(4):
            nc.sync.dma_start(out=wk_sb[i][:], in_=wk[128 * i:128 * i + 128, :])
            nc.sync.dma_start(out=wq_sb[i][:], in_=wq[128 * i:128 * i + 128, :])
        cos_sb = [pers.tile([128, T], BF16, name=f"cos{i}", tag=f"cos{i}") for i in range(2)]
        sin_sb = [pers.tile([128, T], BF16, name=f"sin{i}", tag=f"sin{i}") for i in range(2)]
        nc.sync.dma_start(out=cos_sb[0][:], in_=cosb[0:128, :])
        nc.sync.dma_start(out=sin_sb[0][:], in_=sinb[0:128, :])
        for i in range(4):
            nc.sync.dma_start(out=xT_sb[i][:, QC:T], in_=xT[128 * i:128 * i + 128, QC:T])
        for i in range(4):
            nc.sync.dma_start(out=wv_sb[i][:], in_=wv[128 * i:128 * i + 128, :])
        nc.sync.dma_start(out=cos_sb[1][:], in_=cosb[128:256, :])
        nc.sync.dma_start(out=sin_sb[1][:], in_=sinb[128:256, :])
        msk_sb = pers.tile([128, 256], BF16, name="msk", tag="msk")
        nc.sync.dma_start(out=msk_sb[:], in_=msk[:, :])
        wp_sb = [pers.tile([128, C], BF16, name=f"wp{i}", tag=f"wp{i}") for i in range(2)]
        for i in range(2):
            nc.sync.dma_start(out=wp_sb[i][:], in_=wp[128 * i:128 * i + 128, :])

        # ---------------- persistent SBUF: intermediates ----------------
        qT_sb = [pers.tile([128, T], BF16, name=f"qT{i}", tag=f"qT{i}") for i in range(2)]
        kT_sb = [pers.tile([128, T], BF16, name=f"kT{i}", tag=f"kT{i}") for i in range(2)]
        vx_sb = [pers.tile([128, HPC * (D + 1)], BF16, name=f"vx{i}", tag=f"vx{i}") for i in range(PT)]
        rnT_sb = [pers.tile([128, T], BF16, name=f"rn{i}", tag=f"rn{i}") for i in range(2)]
        # raw (unnormalized) PV^T staging incl. denominator row 64, per (ph, sub)
        raw_sb = [
            [pers.tile([65, T], BF16, name=f"raw{p}{s}", tag=f"raw{p}{s}") for s in range(2)]
            for p in range(2)
        ]
        ones_sb = pers.tile([128, 64], BF16, name="ones", tag="ones")
        nc.vector.memset(ones_sb[:], 1.0)
        for tt in range(PT):
            v3 = vx_sb[tt][:, :].rearrange("p (h x) -> p h x", h=HPC)
            nc.vector.memset(v3[:, :, 64:65], 1.0)

        # ---------------- building blocks ----------------
        def proj_rope(m, which, t4):
            """Project+RoPE one [128, 512] tile of q or k for head pair m."""
            wn = wq_sb if which == "q" else wk_sb
            dst = qT_sb if which == "q" else kT_sb
            tsl = slice(QC * t4, QC * t4 + QC)
            pq = b1.tile([128, 512], F32, name="b1", tag="b1")
            for kc in range(4):
                nc.tensor.matmul(
                    pq[:],
                    lhsT=wn[kc][:, 128 * m:128 * m + 128],
                    rhs=xT_sb[kc][:, tsl],
                    start=(kc == 0),
                    stop=(kc == 3),
                )
            t1 = work.tile([128, 512], BF16, name="t1", tag="t1")
            t2s = work.tile([128, 512], BF16, name="t2s", tag="t2s")
            m1t = work.tile([128, 512], BF16, name="m1t", tag="m1t")
            t2 = work.tile([128, 512], BF16, name="t2", tag="t2")
            nc.scalar.copy(t1[:], pq[:])
            nc.vector.stream_shuffle(t2s[:], t1[:], SWAP_MASK)
            # head-pair 0 feeds attention promptly: keep on DVE. head-pair 1
            # is slack-filled during attention: use GPSIMD.
            eng = nc.vector if m == 0 else nc.gpsimd
            eng.tensor_mul(m1t[:], t1[:], cos_sb[m][:, tsl])
            eng.tensor_mul(t2[:], t2s[:], sin_sb[m][:, tsl])
            eng.tensor_add(dst[m][:, tsl], m1t[:], t2[:])

        def vproj(tt):
            """V projection for one 128-row t tile, interleaved (V|1) layout."""
            pv = b1.tile([128, 512], F32, name="b1", tag="b1")
            for kc in range(4):
                nc.tensor.matmul(
                    pv[:, 0:CL],
                    lhsT=xT_sb[kc][:, 128 * tt:128 * tt + 128],
                    rhs=wv_sb[kc][:],
                    start=(kc == 0),
                    stop=(kc == 3),
                )
            v3 = vx_sb[tt][:, :].rearrange("p (h x) -> p h x", h=HPC)
            p3 = pv[:, 0:CL].rearrange("p (h x) -> p h x", h=HPC)
            nc.vector.tensor_copy(v3[:, :, 0:64], p3[:, :, :])

        def attn_group(ph, j, it, pvp):
            """One attention group: kt tile `it`, both heads of pair ph."""
            nkt = 4 * (j + 1)
            r = it - 4 * j
            lo = 128 * r if r >= 0 else 0   # staircase column offset
            qsl = slice(QC * j + lo, QC * j + QC)
            sg = psc.tile([128, 1024], F32, name="sg", tag="sg")
            for sub in range(2):
                po = 64 * sub
                nc.tensor.matmul(
                    sg[:, 512 * sub + lo:512 * sub + 512],
                    lhsT=kT_sb[ph][po:po + 64, 128 * it:128 * it + 128],
                    rhs=qT_sb[ph][po:po + 64, qsl],
                    start=True,
                    stop=True,
                )
            pg = pexp.tile([128, 1024], BF16, name="pg", tag="pg")
            sg3 = sg[:, :].rearrange("p (b n) -> p b n", b=2)
            pg3 = pg[:, :].rearrange("p (b n) -> p b n", b=2)
            nc.scalar.activation(
                pg3[:, :, lo:512], sg3[:, :, lo:512], ACT_EXP, scale=0.125
            )
            if r >= 0:
                m3 = msk_sb[:, :].rearrange("p (b n) -> p b n", b=2)
                nc.vector.tensor_mul(
                    pg3[:, :, lo:lo + 128], pg3[:, :, lo:lo + 128], m3[:, :, :]
                )
            for sub in range(2):
                h = 2 * ph + sub
                nc.tensor.matmul(
                    pvp[sub][0:65, lo:512],
                    lhsT=vx_sb[it][:, 65 * h:65 * h + 65],
                    rhs=pg[:, 512 * sub + lo:512 * sub + 512],
                    start=(it == 0),
                    stop=(it == nkt - 1),
                )

        def stage_pv(ph, j, pvp):
            """Evacuate PV psum (incl. den row 64) to bf16 staging."""
            qsl = slice(QC * j, QC * j + QC)
            for sub in range(2):
                nc.vector.tensor_copy(raw_sb[ph][sub][:, qsl], pvp[sub][0:65, :])

        def normalize(ph, j):
            """Deferred: broadcast staged den row, reciprocal in psum, scale."""
            qsl = slice(QC * j, QC * j + QC)
            bc = bcp.tile([128, 512], F32, name="bc", tag="bc")
            for sub in range(2):
                nc.tensor.matmul(
                    bc[64 * sub:64 * sub + 64, :],
                    lhsT=ones_sb[64:65, :],
                    rhs=raw_sb[ph][sub][64:65, qsl],
                    start=True,
                    stop=True,
                    tile_position=(64, 64 * sub),
                )
            nc.vector.reciprocal_approx_fast(bc[:], bc[:])
            for sub in range(2):
                # SBUF x PSUM mixed operands: differing base partitions OK
                nc.vector.tensor_mul(
                    rnT_sb[ph][64 * sub:64 * sub + 64, qsl],
                    raw_sb[ph][sub][0:64, qsl],
                    bc[64 * sub:64 * sub + 64, :],
                )

        def proj_out(tt):
            """Output projection for one 128-row t tile + store."""
            pp = b1.tile([128, 512], F32, name="b1", tag="b1")
            for kc in range(2):
                nc.tensor.matmul(
                    pp[:],
                    lhsT=rnT_sb[kc][:, 128 * tt:128 * tt + 128],
                    rhs=wp_sb[kc][:],
                    start=(kc == 0),
                    stop=(kc == 1),
                )
            ys = work.tile([128, 512], BF16, name="ys", tag="ys")
            nc.vector.tensor_copy(ys[:], pp[:])
            nc.sync.dma_start(out=y[128 * tt:128 * tt + 128, :], in_=ys[:])

        # ---------------- schedule ----------------
        # prefix: minimal m0 projections to start attention j=0
        proj_rope(0, "k", 0)
        proj_rope(0, "q", 0)
        for tt in range(4):
            vproj(tt)

        # m1 projections are pure fillers, interleaved into the ACT-bound
        # attention ph=0 phase (all emitted before ph=1 needs them).
        fillers = []
        for t4 in range(NJ):
            fillers.append(("k", t4))
            fillers.append(("q", t4))
        fi = 0

        def emit_fillers(n):
            nonlocal fi
            for _ in range(n):
                if fi >= len(fillers):
                    return
                which, t4 = fillers[fi]
                fi += 1
                proj_rope(1, which, t4)

        for ph in range(2):
            if ph == 1:
                emit_fillers(len(fillers))
            for j in range(NJ):
                if ph == 0 and j >= 1:
                    # projections this j depends on must be emitted first
                    proj_rope(0, "k", j)
                    proj_rope(0, "q", j)
                    for tt in range(4 * j, 4 * j + 4):
                        vproj(tt)
                nkt = 4 * (j + 1)
                pvp = [
                    b1.tile([128, 512], F32, name="b1", tag="b1")
                    for _ in range(2)
                ]
                for it in range(nkt):
                    attn_group(ph, j, it, pvp)
                    if ph == 0 and it % 3 == 2:
                        emit_fillers(1)
                stage_pv(ph, j, pvp)
                # deferred normalizes overlap later attention:
                if ph == 1:
                    normalize(0, j)
                    if j >= 1:
                        normalize(1, j - 1)
                        for tt in range(4 * (j - 1), 4 * j):
                            proj_out(tt)
        normalize(1, NJ - 1)
        for tt in range(4 * (NJ - 1), 4 * NJ):
            proj_out(tt)

    nc.finalize()
    return nc


def prep_core_inputs(x, Wq, Wk, Wv, Wp, core, T):
    b, g = core // 2, core % 2
    sl = slice(CL * g, CL * g + CL)
    lc = np.arange(CL)
    gpair = (CL * g + lc) // 2
    invf = THETA ** (-(2.0 * gpair) / C)
    ang = np.arange(T)[None, :] * invf[:, None]
    cosb = np.cos(ang).astype(np.float32)
    sgn = np.where(lc % 2 == 0, -1.0, 1.0)
    sinb = (np.sin(ang) * sgn[:, None]).astype(np.float32)
    # triangular keep-mask (q >= p) duplicated for the two packed heads
    p = np.arange(128)[:, None]
    q = np.arange(128)[None, :]
    tri = (q >= p).astype(np.float32)
    m = np.concatenate([tri, tri], axis=1)
    return {
        "xT": np.ascontiguousarray(x[b].T).astype(NPBF),
        "wq": np.ascontiguousarray(Wq[sl, :].T).astype(NPBF),
        "wk": np.ascontiguousarray(Wk[sl, :].T).astype(NPBF),
        "wv": np.ascontiguousarray(Wv[sl, :].T).astype(NPBF),
        "wp": np.ascontiguousarray(Wp[:, sl].T).astype(NPBF),
        "cosb": cosb.astype(NPBF),
        "sinb": sinb.astype(NPBF),
        "msk": m.astype(NPBF),
    }


_NC_CACHE = {}


def _get_nc(T):
    if T not in _NC_CACHE:
        _NC_CACHE[T] = build_nc(T)
    return _NC_CACHE[T]


def kernel(x, Wq, Wk, Wv, Wp, bp, _trace=False):
    x = np.asarray(x, dtype=np.float32)
    Wq = np.asarray(Wq, dtype=np.float32)
    Wk = np.asarray(Wk, dtype=np.float32)
    Wv = np.asarray(Wv, dtype=np.float32)
    Wp = np.asarray(Wp, dtype=np.float32)
    bp = np.asarray(bp, dtype=np.float32)
    T = x.shape[1]
    nc = _get_nc(T)
    in_maps = [prep_core_inputs(x, Wq, Wk, Wv, Wp, c, T) for c in range(NCORES)]
    if _trace:
        _ensure_ntff_hook()
    res = run_bass_kernel_spmd(nc, in_maps, list(range(NCORES)), trace=_trace)
    out = np.zeros((B, T, C), np.float32)
    for b in range(B):
        out[b] = res.results[2 * b]["y"].astype(np.float32) + res.results[
            2 * b + 1
        ]["y"].astype(np.float32)
    out += bp[None, None, :]
    if _trace:
        return out, res
    return out


# revision 11
# speedup vs baseline: 1.4777x; 1.2389x over previous
"""Trainium2 Bass kernel for nn_Attention (B=4,T=2048,C=512,H=8 causal RoPE attention).

Sharding: 8 cores = 4 batches x 2 head-groups. Core c handles batch c//2 and
heads [4*(c%2), 4*(c%2)+4). Each core computes its proj partial y_part[T, C]
in bf16; the host sums the two partials per batch (f32) and adds bp.

v3 design (ACT-exp-bound pipeline, engine-balanced):
  - qT = Wq_loc @ x^T; pair-swapped copy via DVE stream_shuffle of the bf16
    cast (cast on ACT); RoPE muls on DVE (head-pair 0, latency-critical) or
    GPSIMD (head-pair 1, slack-filled).
  - Scores transposed S^T[kt, qt], TWO heads per psum group [128, 1024]:
    h0 cols 0:512, h1 cols 512:1024 via row-tiled concurrent matmuls (K=64
    at PE rows 0/64). Causal staircase: diagonal kt-tile r computes only
    qt cols >= 128r; one strided exp [128, 2, 512-128r] covers both heads;
    triangular mask multiply on DVE.
  - PV: (V|1)-stationary M=65 matmuls accumulate out^T + denominator row.
    pvp psum evacuated immediately via one [65,512] cast per sub into bf16
    staging (rawA/rawB); normalize (ones-broadcast matmul of the staged den
    row + reciprocal + mul) is deferred and overlapped with later attention.
  - v-proj / head-pair-1 projections / output proj interleaved into the
    ACT-bound attention phase; bf16 output DMA.
"""

import sys

for _p in ("/opt/trn_rl_repo",):
    if _p not in sys.path:
        sys.path.insert(0, _p)

from contextlib import ExitStack

import ml_dtypes
import numpy as np

import concourse.bass as bass
import concourse.tile as tile
from concourse import bacc
from concourse import mybir
from concourse.bass_utils import run_bass_kernel_spmd


def _ensure_ntff_hook():
    """Provide antenv.axon_hooks (missing in this image) so trace=True works."""
    try:
        import antenv.axon_hooks  # noqa: F401

        return
    except ImportError:
        pass
    import contextlib
    import ctypes
    import types

    import antenv

    mod = types.ModuleType("antenv.axon_hooks")
    holder = {}
    mod.set_axon_ntff_profile_hook = lambda h: holder.__setitem__("h", h)
    mod.get_axon_ntff_profile_hook = lambda: holder.get("h")
    antenv.axon_hooks = mod
    sys.modules["antenv.axon_hooks"] = mod

    so_path = "/opt/axon/libaxon_pjrt.so"
    try:
        lib = ctypes.CDLL(so_path)
    except OSError:
        return
    if not hasattr(lib, "axon_start_nrt_profile"):
        return
    lib.axon_start_nrt_profile.argtypes = [
        ctypes.POINTER(ctypes.c_int64),
        ctypes.c_size_t,
    ]
    lib.axon_start_nrt_profile.restype = ctypes.c_int64
    lib.axon_stop_nrt_profile.argtypes = [ctypes.c_char_p]
    lib.axon_stop_nrt_profile.restype = ctypes.c_int64

    @contextlib.contextmanager
    def _hook(output_dir, device_ids):
        import jax

        jax.devices()
        if device_ids:
            ids = (ctypes.c_int64 * len(device_ids))(*device_ids)
            rc = lib.axon_start_nrt_profile(ids, len(device_ids))
        else:
            rc = lib.axon_start_nrt_profile(None, 0)
        if rc != 0:
            raise RuntimeError(f"axon_start_nrt_profile rc={rc}")
        try:
            yield
        finally:
            n = lib.axon_stop_nrt_profile(str(output_dir).encode())
            print(f"profile: {n} file(s) written to {output_dir}", file=sys.stderr)

    mod.set_axon_ntff_profile_hook(_hook)


BF16 = mybir.dt.bfloat16
F32 = mybir.dt.float32
NPBF = ml_dtypes.bfloat16

B, C, H, D = 4, 512, 8, 64
HPC = 4              # heads per core
CL = HPC * D         # 256 local channels
NCORES = 8
THETA = 10000.0
QC = 512             # q-chunk width
ACT_EXP = mybir.ActivationFunctionType.Exp

SWAP_MASK = [i ^ 1 for i in range(32)]


def build_nc(T: int) -> bass.Bass:
    PT = T // 128
    NJ = T // QC
    nc = bacc.Bacc()

    xT = nc.declare_dram_parameter("xT", [C, T], BF16, isOutput=False)
    wq = nc.declare_dram_parameter("wq", [C, CL], BF16, isOutput=False)
    wk = nc.declare_dram_parameter("wk", [C, CL], BF16, isOutput=False)
    wv = nc.declare_dram_parameter("wv", [C, CL], BF16, isOutput=False)
    wp = nc.declare_dram_parameter("wp", [CL, C], BF16, isOutput=False)
    cosb = nc.declare_dram_parameter("cosb", [CL, T], BF16, isOutput=False)
    sinb = nc.declare_dram_parameter("sinb", [CL, T], BF16, isOutput=False)
    msk = nc.declare_dram_parameter("msk", [128, 256], BF16, isOutput=False)
    y = nc.declare_dram_parameter("y", [T, C], BF16, isOutput=True)

    with nc.allow_low_precision(
        reason="bf16 compute by design; f32 PSUM accumulation everywhere"
    ), tile.TileContext(nc) as tc, ExitStack() as ctx:
        pers = ctx.enter_context(tc.tile_pool(name="pers", bufs=1))
        work = ctx.enter_context(tc.tile_pool(name="work", bufs=4))
        pexp = ctx.enter_context(tc.tile_pool(name="pexp", bufs=6))
        psc = ctx.enter_context(tc.tile_pool(name="psc", bufs=2, space="PSUM"))
        b1 = ctx.enter_context(tc.tile_pool(name="b1", bufs=3, space="PSUM"))
        bcp = ctx.enter_context(tc.tile_pool(name="bcp", bufs=1, space="PSUM"))

        # ---------------- persistent SBUF: inputs ----------------
        xT_sb = [pers.tile([128, T], BF16, name=f"xT{i}", tag=f"xT{i}") for i in range(4)]

        def wtiles(name):
            return [
                pers.tile([128, CL], BF16, name=f"{name}{i}", tag=f"{name}{i}")
                for i in range(4)
            ]

        wq_sb, wk_sb, wv_sb = wtiles("wq"), wtiles("wk"), wtiles("wv")
        # first q-chunk of x lands first so the first projections start early
        for i in range(4):
            nc.sync.dma_start(out=xT_sb[i][:, 0:QC], in_=xT[128 * i:128 * i + 128, 0:QC])
        for i in range(4):
            nc.sync.dma_start(out=wk_sb[i][:], in_=wk[128 * i:128 * i + 128, :])
            nc.sync.dma_start(out=wq_sb[i][:], in_=wq[128 * i:128 * i + 128, :])
        cos_sb = [pers.tile([128, T], BF16, name=f"cos{i}", tag=f"cos{i}") for i in range(2)]
        sin_sb = [pers.tile([128, T], BF16, name=f"sin{i}", tag=f"sin{i}") for i in range(2)]
        nc.sync.dma_start(out=cos_sb[0][:], in_=cosb[0:128, :])
        nc.sync.dma_start(out=sin_sb[0][:], in_=sinb[0:128, :])
        for i in range(4):
            nc.sync.dma_start(out=xT_sb[i][:, QC:T], in_=xT[128 * i:128 * i + 128, QC:T])
        for i in range(4):
            nc.sync.dma_start(out=wv_sb[i][:], in_=wv[128 * i:128 * i + 128, :])
        nc.sync.dma_start(out=cos_sb[1][:], in_=cosb[128:256, :])
        nc.sync.dma_start(out=sin_sb[1][:], in_=sinb[128:256, :])
        msk_sb = pers.tile([128, 256], BF16, name="msk", tag="msk")
        nc.sync.dma_start(out=msk_sb[:], in_=msk[:, :])
        wp_sb = [pers.tile([128, C], BF16, name=f"wp{i}", tag=f"wp{i}") for i in range(2)]
        for i in range(2):
            nc.sync.dma_start(out=wp_sb[i][:], in_=wp[128 * i:128 * i + 128, :])

        # ---------------- persistent SBUF: intermediates ----------------
        qT_sb = [pers.tile([128, T], BF16, name=f"qT{i}", tag=f"qT{i}") for i in range(2)]
        kT_sb = [pers.tile([128, T], BF16, name=f"kT{i}", tag=f"kT{i}") for i in range(2)]
        vx_sb = [pers.tile([128, HPC * (D + 1)], BF16, name=f"vx{i}", tag=f"vx{i}") for i in range(PT)]
        rnT_sb = [pers.tile([128, T], BF16, name=f"rn{i}", tag=f"rn{i}") for i in range(2)]
        # raw (unnormalized) PV^T staging incl. denominator row 64, per (ph, sub)
        raw_sb = [
            [pers.tile([65, T], BF16, name=f"raw{p}{s}", tag=f"raw{p}{s}") for s in range(2)]
            for p in range(2)
        ]
        ones_sb = pers.tile([128, 64], BF16, name="ones", tag="ones")
        nc.vector.memset(ones_sb[:], 1.0)
        for tt in range(PT):
            v3 = vx_sb[tt][:, :].rearrange("p (h x) -> p h x", h=HPC)
            nc.vector.memset(v3[:, :, 64:65], 1.0)

        # ---------------- building blocks ----------------
        def proj_rope(m, which, t4):
            """Project+RoPE one [128, 512] tile of q or k for head pair m."""
            wn = wq_sb if which == "q" else wk_sb
            dst = qT_sb if which == "q" else kT_sb
            tsl = slice(QC * t4, QC * t4 + QC)
            pq = b1.tile([128, 512], F32, name="b1", tag="b1")
            for kc in range(4):
                nc.tensor.matmul(
                    pq[:],
                    lhsT=wn[kc][:, 128 * m:128 * m + 128],
                    rhs=xT_sb[kc][:, tsl],
                    start=(kc == 0),
                    stop=(kc == 3),
                )
            t1 = work.tile([128, 512], BF16, name="t1", tag="t1")
            t2s = work.tile([128, 512], BF16, name="t2s", tag="t2s")
            m1t = work.tile([128, 512], BF16, name="m1t", tag="m1t")
            t2 = work.tile([128, 512], BF16, name="t2", tag="t2")
            nc.scalar.copy(t1[:], pq[:])
            nc.vector.stream_shuffle(t2s[:], t1[:], SWAP_MASK)
            # head-pair 0 feeds attention promptly: keep on DVE. head-pair 1
            # is slack-filled during attention: use GPSIMD.
            eng = nc.vector if m == 0 else nc.gpsimd
            eng.tensor_mul(m1t[:], t1[:], cos_sb[m][:, tsl])
            eng.tensor_mul(t2[:], t2s[:], sin_sb[m][:, tsl])
            eng.tensor_add(dst[m][:, tsl], m1t[:], t2[:])

        def vproj(tt):
            """V projection for one 128-row t tile, interleaved (V|1) layout."""
            pv = b1.tile([128, 512], F32, name="b1", tag="b1")
            for kc in range(4):
                nc.tensor.matmul(
                    pv[:, 0:CL],
                    lhsT=xT_sb[kc][:, 128 * tt:128 * tt + 128],
                    rhs=wv_sb[kc][:],
                    start=(kc == 0),
                    stop=(kc == 3),
                )
            v3 = vx_sb[tt][:, :].rearrange("p (h x) -> p h x", h=HPC)
            p3 = pv[:, 0:CL].rearrange("p (h x) -> p h x", h=HPC)
            nc.vector.tensor_copy(v3[:, :, 0:64], p3[:, :, :])

        def attn_scores(ph, j, it):
            """Scores + exp + mask for kt tile `it`, both heads of pair ph.
            Returns the pg tile for the deferred PV step."""
            r = it - 4 * j
            lo = 128 * r if r >= 0 else 0   # staircase column offset
            qsl = slice(QC * j + lo, QC * j + QC)
            sg = psc.tile([128, 1024], F32, name="sg", tag="sg")
            for sub in range(2):
                po = 64 * sub
                nc.tensor.matmul(
                    sg[:, 512 * sub + lo:512 * sub + 512],
                    lhsT=kT_sb[ph][po:po + 64, 128 * it:128 * it + 128],
                    rhs=qT_sb[ph][po:po + 64, qsl],
                    start=True,
                    stop=True,
                )
            pg = pexp.tile([128, 1024], BF16, name="pg", tag="pg")
            sg3 = sg[:, :].rearrange("p (b n) -> p b n", b=2)
            pg3 = pg[:, :].rearrange("p (b n) -> p b n", b=2)
            nc.scalar.activation(
                pg3[:, :, lo:512], sg3[:, :, lo:512], ACT_EXP, scale=0.125
            )
            if r >= 0:
                m3 = msk_sb[:, :].rearrange("p (b n) -> p b n", b=2)
                nc.vector.tensor_mul(
                    pg3[:, :, lo:lo + 128], pg3[:, :, lo:lo + 128], m3[:, :, :]
                )
            return pg, lo

        def attn_pv(ph, j, it, pvp, pg, lo):
            """PV accumulation for a previously emitted scores group."""
            nkt = 4 * (j + 1)
            for sub in range(2):
                h = 2 * ph + sub
                nc.tensor.matmul(
                    pvp[sub][0:65, lo:512],
                    lhsT=vx_sb[it][:, 65 * h:65 * h + 65],
                    rhs=pg[:, 512 * sub + lo:512 * sub + 512],
                    start=(it == 0),
                    stop=(it == nkt - 1),
                )

        def stage_pv(ph, j, pvp):
            """Evacuate PV psum (incl. den row 64) to bf16 staging."""
            qsl = slice(QC * j, QC * j + QC)
            for sub in range(2):
                nc.vector.tensor_copy(raw_sb[ph][sub][:, qsl], pvp[sub][0:65, :])

        def normalize(ph, j):
            """Deferred: broadcast staged den row, reciprocal in psum, scale."""
            qsl = slice(QC * j, QC * j + QC)
            bc = bcp.tile([128, 512], F32, name="bc", tag="bc")
            for sub in range(2):
                nc.tensor.matmul(
                    bc[64 * sub:64 * sub + 64, :],
                    lhsT=ones_sb[64:65, :],
                    rhs=raw_sb[ph][sub][64:65, qsl],
                    start=True,
                    stop=True,
                    tile_position=(64, 64 * sub),
                )
            nc.vector.reciprocal_approx_fast(bc[:], bc[:])
            for sub in range(2):
                # SBUF x PSUM mixed operands: differing base partitions OK
                nc.vector.tensor_mul(
                    rnT_sb[ph][64 * sub:64 * sub + 64, qsl],
                    raw_sb[ph][sub][0:64, qsl],
                    bc[64 * sub:64 * sub + 64, :],
                )

        def proj_out(tt):
            """Output projection for one 128-row t tile + store."""
            pp = b1.tile([128, 512], F32, name="b1", tag="b1")
            for kc in range(2):
                nc.tensor.matmul(
                    pp[:],
                    lhsT=rnT_sb[kc][:, 128 * tt:128 * tt + 128],
                    rhs=wp_sb[kc][:],
                    start=(kc == 0),
                    stop=(kc == 1),
                )
            ys = work.tile([128, 512], BF16, name="ys", tag="ys")
            nc.vector.tensor_copy(ys[:], pp[:])
            nc.sync.dma_start(out=y[128 * tt:128 * tt + 128, :], in_=ys[:])

        # ---------------- schedule ----------------
        # prefix: minimal projections to start attention (ph0, j0)
        proj_rope(0, "k", 0)
        proj_rope(0, "q", 0)
        for tt in range(4):
            vproj(tt)

        # remaining projections are fillers drip-fed into the attention
        # phase; hard deadlines enforced via drain_until.
        fillers = []
        for j in range(1, NJ):
            fillers.append((f"k{j}", ("r", 0, "k", j)))
            fillers.append((f"q{j}", ("r", 0, "q", j)))
            for tt in range(4 * j, 4 * j + 4):
                fillers.append((f"v{tt}", ("v", tt)))
        for t4 in range(NJ):
            fillers.append((f"K{t4}", ("r", 1, "k", t4)))
            fillers.append((f"Q{t4}", ("r", 1, "q", t4)))
        fi = 0
        emitted = set()

        def emit_filler():
            nonlocal fi
            if fi >= len(fillers):
                return
            key, spec = fillers[fi]
            fi += 1
            emitted.add(key)
            if spec[0] == "v":
                vproj(spec[1])
            else:
                proj_rope(spec[1], spec[2], spec[3])

        def drain_until(key):
            while key not in emitted and fi < len(fillers):
                emit_filler()

        # flat software-pipelined group stream: scores of group g+1 are
        # emitted BEFORE the PV of group g so the PE FIFO never stalls the
        # exp stream on the exp->mask->PV round trip.
        groups = [
            (ph, j, it)
            for ph in range(2)
            for j in range(NJ)
            for it in range(4 * (j + 1))
        ]
        pvps = {}
        pend = None

        def get_pvp(ph, j):
            if (ph, j) not in pvps:
                pvps[(ph, j)] = [
                    b1.tile([128, 512], F32, name="b1", tag="b1")
                    for _ in range(2)
                ]
            return pvps[(ph, j)]

        def post_j(ph, j):
            stage_pv(ph, j, pvps.pop((ph, j)))
            if ph == 1:
                normalize(0, j)
                if j >= 1:
                    normalize(1, j - 1)
                    for tt in range(4 * (j - 1), 4 * j):
                        proj_out(tt)

        for ph, j, it in groups:
            if it == 0:
                if ph == 0 and j >= 1:
                    drain_until(f"k{j}")
                    drain_until(f"q{j}")
                if ph == 1 and j == 0:
                    drain_until(f"Q{NJ - 1}")
            if ph == 0 and it >= 4:
                drain_until(f"v{it}")
            pg, lo = attn_scores(ph, j, it)
            if pend is not None:
                pph, pj, pit, ppg, plo = pend
                attn_pv(pph, pj, pit, get_pvp(pph, pj), ppg, plo)
                if pit == 4 * (pj + 1) - 1:
                    post_j(pph, pj)
            if ph == 0:
                emit_filler()
            pend = (ph, j, it, pg, lo)
        pph, pj, pit, ppg, plo = pend
        attn_pv(pph, pj, pit, get_pvp(pph, pj), ppg, plo)
        post_j(pph, pj)
        normalize(1, NJ - 1)
        for tt in range(4 * (NJ - 1), 4 * NJ):
            proj_out(tt)

    nc.finalize()
    return nc


def prep_core_inputs(x, Wq, Wk, Wv, Wp, core, T):
    b, g = core // 2, core % 2
    sl = slice(CL * g, CL * g + CL)
    lc = np.arange(CL)
    gpair = (CL * g + lc) // 2
    invf = THETA ** (-(2.0 * gpair) / C)
    ang = np.arange(T)[None, :] * invf[:, None]
    cosb = np.cos(ang).astype(np.float32)
    sgn = np.where(lc % 2 == 0, -1.0, 1.0)
    sinb = (np.sin(ang) * sgn[:, None]).astype(np.float32)
    # triangular keep-mask (q >= p) duplicated for the two packed heads
    p = np.arange(128)[:, None]
    q = np.arange(128)[None, :]
    tri = (q >= p).astype(np.float32)
    m = np.concatenate([tri, tri], axis=1)
    return {
        "xT": np.ascontiguousarray(x[b].T).astype(NPBF),
        "wq": np.ascontiguousarray(Wq[sl, :].T).astype(NPBF),
        "wk": np.ascontiguousarray(Wk[sl, :].T).astype(NPBF),
        "wv": np.ascontiguousarray(Wv[sl, :].T).astype(NPBF),
        "wp": np.ascontiguousarray(Wp[:, sl].T).astype(NPBF),
        "cosb": cosb.astype(NPBF),
        "sinb": sinb.astype(NPBF),
        "msk": m.astype(NPBF),
    }


_NC_CACHE = {}


def _get_nc(T):
    if T not in _NC_CACHE:
        _NC_CACHE[T] = build_nc(T)
    return _NC_CACHE[T]


def kernel(x, Wq, Wk, Wv, Wp, bp, _trace=False):
    x = np.asarray(x, dtype=np.float32)
    Wq = np.asarray(Wq, dtype=np.float32)
    Wk = np.asarray(Wk, dtype=np.float32)
    Wv = np.asarray(Wv, dtype=np.float32)
    Wp = np.asarray(Wp, dtype=np.float32)
    bp = np.asarray(bp, dtype=np.float32)
    T = x.shape[1]
    nc = _get_nc(T)
    in_maps = [prep_core_inputs(x, Wq, Wk, Wv, Wp, c, T) for c in range(NCORES)]
    if _trace:
        _ensure_ntff_hook()
    res = run_bass_kernel_spmd(nc, in_maps, list(range(NCORES)), trace=_trace)
    out = np.zeros((B, T, C), np.float32)
    for b in range(B):
        out[b] = res.results[2 * b]["y"].astype(np.float32) + res.results[
            2 * b + 1
        ]["y"].astype(np.float32)
    out += bp[None, None, :]
    if _trace:
        return out, res
    return out


# revision 17
# speedup vs baseline: 1.5230x; 1.0306x over previous
"""Trainium2 Bass kernel for nn_Attention (B=4,T=2048,C=512,H=8 causal RoPE attention).

Sharding: 8 cores = 4 batches x 2 head-groups. Core c handles batch c//2 and
heads [4*(c%2), 4*(c%2)+4). Each core computes its proj partial y_part[T, C]
in bf16; the host sums the two partials per batch (f32) and adds bp.

v3 design (ACT-exp-bound pipeline, engine-balanced):
  - qT = Wq_loc @ x^T; pair-swapped copy via DVE stream_shuffle of the bf16
    cast (cast on ACT); RoPE muls on DVE (head-pair 0, latency-critical) or
    GPSIMD (head-pair 1, slack-filled).
  - Scores transposed S^T[kt, qt], TWO heads per psum group [128, 1024]:
    h0 cols 0:512, h1 cols 512:1024 via row-tiled concurrent matmuls (K=64
    at PE rows 0/64). Causal staircase: diagonal kt-tile r computes only
    qt cols >= 128r; one strided exp [128, 2, 512-128r] covers both heads;
    triangular mask multiply on DVE.
  - PV: (V|1)-stationary M=65 matmuls accumulate out^T + denominator row.
    pvp psum evacuated immediately via one [65,512] cast per sub into bf16
    staging (rawA/rawB); normalize (ones-broadcast matmul of the staged den
    row + reciprocal + mul) is deferred and overlapped with later attention.
  - v-proj / head-pair-1 projections / output proj interleaved into the
    ACT-bound attention phase; bf16 output DMA.
"""

import sys

for _p in ("/opt/trn_rl_repo",):
    if _p not in sys.path:
        sys.path.insert(0, _p)

from contextlib import ExitStack

import ml_dtypes
import numpy as np

import concourse.bass as bass
import concourse.tile as tile
from concourse import bacc
from concourse import mybir
from concourse.bass_utils import run_bass_kernel_spmd


def _ensure_ntff_hook():
    """Provide antenv.axon_hooks (missing in this image) so trace=True works."""
    try:
        import antenv.axon_hooks  # noqa: F401

        return
    except ImportError:
        pass
    import contextlib
    import ctypes
    import types

    import antenv

    mod = types.ModuleType("antenv.axon_hooks")
    holder = {}
    mod.set_axon_ntff_profile_hook = lambda h: holder.__setitem__("h", h)
    mod.get_axon_ntff_profile_hook = lambda: holder.get("h")
    antenv.axon_hooks = mod
    sys.modules["antenv.axon_hooks"] = mod

    so_path = "/opt/axon/libaxon_pjrt.so"
    try:
        lib = ctypes.CDLL(so_path)
    except OSError:
        return
    if not hasattr(lib, "axon_start_nrt_profile"):
        return
    lib.axon_start_nrt_profile.argtypes = [
        ctypes.POINTER(ctypes.c_int64),
        ctypes.c_size_t,
    ]
    lib.axon_start_nrt_profile.restype = ctypes.c_int64
    lib.axon_stop_nrt_profile.argtypes = [ctypes.c_char_p]
    lib.axon_stop_nrt_profile.restype = ctypes.c_int64

    @contextlib.contextmanager
    def _hook(output_dir, device_ids):
        import jax

        jax.devices()
        if device_ids:
            ids = (ctypes.c_int64 * len(device_ids))(*device_ids)
            rc = lib.axon_start_nrt_profile(ids, len(device_ids))
        else:
            rc = lib.axon_start_nrt_profile(None, 0)
        if rc != 0:
            raise RuntimeError(f"axon_start_nrt_profile rc={rc}")
        try:
            yield
        finally:
            n = lib.axon_stop_nrt_profile(str(output_dir).encode())
            print(f"profile: {n} file(s) written to {output_dir}", file=sys.stderr)

    mod.set_axon_ntff_profile_hook(_hook)


BF16 = mybir.dt.bfloat16
F32 = mybir.dt.float32
NPBF = ml_dtypes.bfloat16

B, C, H, D = 4, 512, 8, 64
HPC = 4              # heads per core
CL = HPC * D         # 256 local channels
NCORES = 8
THETA = 10000.0
QC = 512             # q-chunk width
ACT_EXP = mybir.ActivationFunctionType.Exp

SWAP_MASK = [i ^ 1 for i in range(32)]


def build_nc(T: int) -> bass.Bass:
    PT = T // 128
    NJ = T // QC
    nc = bacc.Bacc()

    # packed layouts: one [128, ...] DRAM tensor per logical input so each
    # loads with a single DMA (the Sync queue issues DMAs serially at
    # ~650ns each; 27 separate tile loads cost ~18us of prefix).
    xT = nc.declare_dram_parameter("xT", [128, 4 * T], BF16, isOutput=False)
    wq = nc.declare_dram_parameter("wq", [128, 4 * CL], BF16, isOutput=False)
    wk = nc.declare_dram_parameter("wk", [128, 4 * CL], BF16, isOutput=False)
    wv = nc.declare_dram_parameter("wv", [128, 4 * CL], BF16, isOutput=False)
    wp = nc.declare_dram_parameter("wp", [128, 2 * C], BF16, isOutput=False)
    # cs: cols [0:T)=cos m0, [T:2T)=cos m1, [2T:3T)=sin m0, [3T:4T)=sin m1
    cs = nc.declare_dram_parameter("cs", [128, 4 * T], BF16, isOutput=False)
    msk = nc.declare_dram_parameter("msk", [128, 256], BF16, isOutput=False)
    y = nc.declare_dram_parameter("y", [T, C], BF16, isOutput=True)

    with nc.allow_low_precision(
        reason="bf16 compute by design; f32 PSUM accumulation everywhere"
    ), tile.TileContext(nc) as tc, ExitStack() as ctx:
        pers = ctx.enter_context(tc.tile_pool(name="pers", bufs=1))
        work = ctx.enter_context(tc.tile_pool(name="work", bufs=4))
        pexp = ctx.enter_context(tc.tile_pool(name="pexp", bufs=6))
        psc = ctx.enter_context(tc.tile_pool(name="psc", bufs=2, space="PSUM"))
        b1 = ctx.enter_context(tc.tile_pool(name="b1", bufs=3, space="PSUM"))
        bcp = ctx.enter_context(tc.tile_pool(name="bcp", bufs=1, space="PSUM"))

        # ---------------- persistent SBUF: inputs ----------------
        xT_sb = pers.tile([128, 4 * T], BF16, name="xT", tag="xT")
        wq_sb = pers.tile([128, 4 * CL], BF16, name="wq", tag="wq")
        wk_sb = pers.tile([128, 4 * CL], BF16, name="wk", tag="wk")
        wv_sb = pers.tile([128, 4 * CL], BF16, name="wv", tag="wv")
        cs_sb = pers.tile([128, 4 * T], BF16, name="cs", tag="cs")
        wp_sb = pers.tile([128, 2 * C], BF16, name="wp", tag="wp")
        msk_sb = pers.tile([128, 256], BF16, name="msk", tag="msk")
        # first q-chunk of x (all 4 k-tiles, strided) lands first so the
        # first projections start early
        x4d = xT[:, :].rearrange("p (k t) -> p k t", k=4)
        x4s = xT_sb[:, :].rearrange("p (k t) -> p k t", k=4)
        nc.sync.dma_start(out=x4s[:, :, 0:QC], in_=x4d[:, :, 0:QC])
        nc.sync.dma_start(out=wk_sb[:], in_=wk[:, :])
        nc.sync.dma_start(out=wq_sb[:], in_=wq[:, :])
        # cs blocks: [cos m0 | cos m1 | sin m0 | sin m1]; s-view pairs
        # {cos,sin} so one strided DMA loads head-pair m's tables.
        c4d = cs[:, :].rearrange("p (s t) -> p s t", s=2)
        c4s = cs_sb[:, :].rearrange("p (s t) -> p s t", s=2)
        nc.sync.dma_start(out=c4s[:, :, 0:T], in_=c4d[:, :, 0:T])       # cos+sin m0
        nc.sync.dma_start(out=x4s[:, :, QC:T], in_=x4d[:, :, QC:T])
        nc.sync.dma_start(out=wv_sb[:], in_=wv[:, :])
        nc.sync.dma_start(out=c4s[:, :, T:2 * T], in_=c4d[:, :, T:2 * T])  # cos+sin m1
        nc.sync.dma_start(out=msk_sb[:], in_=msk[:, :])
        nc.sync.dma_start(out=wp_sb[:], in_=wp[:, :])

        # ---------------- persistent SBUF: intermediates ----------------
        qT_sb = [pers.tile([128, T], BF16, name=f"qT{i}", tag=f"qT{i}") for i in range(2)]
        kT_sb = [pers.tile([128, T], BF16, name=f"kT{i}", tag=f"kT{i}") for i in range(2)]
        vx_sb = [pers.tile([128, HPC * (D + 1)], BF16, name=f"vx{i}", tag=f"vx{i}") for i in range(PT)]
        rnT_sb = [pers.tile([128, T], BF16, name=f"rn{i}", tag=f"rn{i}") for i in range(2)]
        # raw (unnormalized) PV^T staging incl. denominator row 64, per (ph, sub)
        raw_sb = [
            [pers.tile([65, T], BF16, name=f"raw{p}{s}", tag=f"raw{p}{s}") for s in range(2)]
            for p in range(2)
        ]
        ones_sb = pers.tile([128, 64], BF16, name="ones", tag="ones")
        nc.vector.memset(ones_sb[:], 1.0)
        for tt in range(PT):
            v3 = vx_sb[tt][:, :].rearrange("p (h x) -> p h x", h=HPC)
            nc.vector.memset(v3[:, :, 64:65], 1.0)

        # ---------------- building blocks ----------------
        def proj_rope(m, which, t4):
            """Project+RoPE one [128, 512] tile of q or k for head pair m."""
            wn = wq_sb if which == "q" else wk_sb
            dst = qT_sb if which == "q" else kT_sb
            tsl = slice(QC * t4, QC * t4 + QC)
            cossl = slice(m * T + QC * t4, m * T + QC * t4 + QC)
            sinsl = slice(2 * T + m * T + QC * t4, 2 * T + m * T + QC * t4 + QC)
            pq = b1.tile([128, 512], F32, name="b1", tag="b1")
            for kc in range(4):
                nc.tensor.matmul(
                    pq[:],
                    lhsT=wn_sl(wn, kc, m),
                    rhs=xT_sb[:, T * kc + QC * t4:T * kc + QC * t4 + QC],
                    start=(kc == 0),
                    stop=(kc == 3),
                )
            t2s = work.tile([128, 512], F32, name="t2s", tag="t2s")
            m1t = work.tile([128, 512], BF16, name="m1t", tag="m1t")
            t2 = work.tile([128, 512], BF16, name="t2", tag="t2")
            nc.vector.stream_shuffle(t2s[:], pq[:], SWAP_MASK)
            nc.vector.tensor_mul(m1t[:], pq[:], cs_sb[:, cossl])
            # head-pair 0 feeds attention promptly: keep on DVE. head-pair 1
            # is slack-filled during attention: use GPSIMD.
            eng = nc.vector if m == 0 else nc.gpsimd
            eng.tensor_mul(t2[:], t2s[:], cs_sb[:, sinsl])
            eng.tensor_add(dst[m][:, tsl], m1t[:], t2[:])

        def wn_sl(wn, kc, m):
            return wn[:, CL * kc + 128 * m:CL * kc + 128 * m + 128]

        def vproj(tt):
            """V projection for one 128-row t tile, interleaved (V|1) layout."""
            pv = b1.tile([128, 512], F32, name="b1", tag="b1")
            for kc in range(4):
                nc.tensor.matmul(
                    pv[:, 0:CL],
                    lhsT=xT_sb[:, T * kc + 128 * tt:T * kc + 128 * tt + 128],
                    rhs=wv_sb[:, CL * kc:CL * kc + CL],
                    start=(kc == 0),
                    stop=(kc == 3),
                )
            v3 = vx_sb[tt][:, :].rearrange("p (h x) -> p h x", h=HPC)
            p3 = pv[:, 0:CL].rearrange("p (h x) -> p h x", h=HPC)
            nc.vector.tensor_copy(v3[:, :, 0:64], p3[:, :, :])

        def attn_scores(ph, j, it):
            """Scores + exp + mask for kt tile `it`, both heads of pair ph.
            Returns the pg tile for the deferred PV step."""
            r = it - 4 * j
            lo = 128 * r if r >= 0 else 0   # staircase column offset
            qsl = slice(QC * j + lo, QC * j + QC)
            sg = psc.tile([128, 1024], F32, name="sg", tag="sg")
            for sub in range(2):
                po = 64 * sub
                nc.tensor.matmul(
                    sg[:, 512 * sub + lo:512 * sub + 512],
                    lhsT=kT_sb[ph][po:po + 64, 128 * it:128 * it + 128],
                    rhs=qT_sb[ph][po:po + 64, qsl],
                    start=True,
                    stop=True,
                )
            pg = pexp.tile([128, 1024], BF16, name="pg", tag="pg")
            sg3 = sg[:, :].rearrange("p (b n) -> p b n", b=2)
            pg3 = pg[:, :].rearrange("p (b n) -> p b n", b=2)
            nc.scalar.activation(
                pg3[:, :, lo:512], sg3[:, :, lo:512], ACT_EXP, scale=0.125
            )
            if r >= 0:
                m3 = msk_sb[:, :].rearrange("p (b n) -> p b n", b=2)
                nc.vector.tensor_mul(
                    pg3[:, :, lo:lo + 128], pg3[:, :, lo:lo + 128], m3[:, :, :]
                )
            return pg, lo

        def attn_pv(ph, j, it, pvp, pg, lo):
            """PV accumulation for a previously emitted scores group."""
            nkt = 4 * (j + 1)
            for sub in range(2):
                h = 2 * ph + sub
                nc.tensor.matmul(
                    pvp[sub][0:65, lo:512],
                    lhsT=vx_sb[it][:, 65 * h:65 * h + 65],
                    rhs=pg[:, 512 * sub + lo:512 * sub + 512],
                    start=(it == 0),
                    stop=(it == nkt - 1),
                )

        def stage_pv(ph, j, pvp):
            """Evacuate PV psum (incl. den row 64) to bf16 staging."""
            qsl = slice(QC * j, QC * j + QC)
            for sub in range(2):
                nc.vector.tensor_copy(raw_sb[ph][sub][:, qsl], pvp[sub][0:65, :])

        def normalize(ph, j):
            """Deferred: broadcast staged den row, reciprocal in psum, scale."""
            qsl = slice(QC * j, QC * j + QC)
            bc = bcp.tile([128, 512], F32, name="bc", tag="bc")
            for sub in range(2):
                nc.tensor.matmul(
                    bc[64 * sub:64 * sub + 64, :],
                    lhsT=ones_sb[64:65, :],
                    rhs=raw_sb[ph][sub][64:65, qsl],
                    start=True,
                    stop=True,
                    tile_position=(64, 64 * sub),
                )
            nc.vector.reciprocal_approx_fast(bc[:], bc[:])
            for sub in range(2):
                # SBUF x PSUM mixed operands: differing base partitions OK
                nc.vector.tensor_mul(
                    rnT_sb[ph][64 * sub:64 * sub + 64, qsl],
                    raw_sb[ph][sub][0:64, qsl],
                    bc[64 * sub:64 * sub + 64, :],
                )

        def proj_out(tt):
            """Output projection for one 128-row t tile + store."""
            pp = b1.tile([128, 512], F32, name="b1", tag="b1")
            for kc in range(2):
                nc.tensor.matmul(
                    pp[:],
                    lhsT=rnT_sb[kc][:, 128 * tt:128 * tt + 128],
                    rhs=wp_sb[:, C * kc:C * kc + C],
                    start=(kc == 0),
                    stop=(kc == 1),
                )
            ys = work.tile([128, 512], BF16, name="ys", tag="ys")
            nc.vector.tensor_copy(ys[:], pp[:])
            nc.sync.dma_start(out=y[128 * tt:128 * tt + 128, :], in_=ys[:])

        # ---------------- schedule ----------------
        # prefix: minimal projections to start attention (ph0, j0)
        proj_rope(0, "k", 0)
        proj_rope(0, "q", 0)
        for tt in range(4):
            vproj(tt)

        # remaining projections are fillers drip-fed into the attention
        # phase; hard deadlines enforced via drain_until.
        fillers = []
        for j in range(1, NJ):
            fillers.append((f"k{j}", ("r", 0, "k", j)))
            fillers.append((f"q{j}", ("r", 0, "q", j)))
            for tt in range(4 * j, 4 * j + 4):
                fillers.append((f"v{tt}", ("v", tt)))
        for t4 in range(NJ):
            fillers.append((f"K{t4}", ("r", 1, "k", t4)))
            fillers.append((f"Q{t4}", ("r", 1, "q", t4)))
        fi = 0
        emitted = set()

        def emit_filler():
            nonlocal fi
            if fi >= len(fillers):
                return
            key, spec = fillers[fi]
            fi += 1
            emitted.add(key)
            if spec[0] == "v":
                vproj(spec[1])
            else:
                proj_rope(spec[1], spec[2], spec[3])

        def drain_until(key):
            while key not in emitted and fi < len(fillers):
                emit_filler()

        # flat software-pipelined group stream: scores of group g+1 are
        # emitted BEFORE the PV of group g so the PE FIFO never stalls the
        # exp stream on the exp->mask->PV round trip.
        groups = [
            (ph, j, it)
            for ph in range(2)
            for j in range(NJ)
            for it in range(4 * (j + 1))
        ]
        pvps = {}
        pend = None

        def get_pvp(ph, j):
            if (ph, j) not in pvps:
                pvps[(ph, j)] = [
                    b1.tile([128, 512], F32, name="b1", tag="b1")
                    for _ in range(2)
                ]
            return pvps[(ph, j)]

        def post_j(ph, j):
            stage_pv(ph, j, pvps.pop((ph, j)))
            if ph == 1:
                normalize(0, j)
                if j >= 1:
                    normalize(1, j - 1)
                    for tt in range(4 * (j - 1), 4 * j):
                        proj_out(tt)

        for ph, j, it in groups:
            if it == 0:
                if ph == 0 and j >= 1:
                    drain_until(f"k{j}")
                    drain_until(f"q{j}")
                if ph == 1 and j == 0:
                    drain_until(f"Q{NJ - 1}")
            if ph == 0 and it >= 4:
                drain_until(f"v{it}")
            pg, lo = attn_scores(ph, j, it)
            if pend is not None:
                pph, pj, pit, ppg, plo = pend
                attn_pv(pph, pj, pit, get_pvp(pph, pj), ppg, plo)
                if pit == 4 * (pj + 1) - 1:
                    post_j(pph, pj)
            if ph == 0:
                emit_filler()
            pend = (ph, j, it, pg, lo)
        pph, pj, pit, ppg, plo = pend
        attn_pv(pph, pj, pit, get_pvp(pph, pj), ppg, plo)
        post_j(pph, pj)
        normalize(1, NJ - 1)
        for tt in range(4 * (NJ - 1), 4 * NJ):
            proj_out(tt)

    nc.finalize()
    return nc


def prep_core_inputs(x, Wq, Wk, Wv, Wp, core, T):
    b, g = core // 2, core % 2
    sl = slice(CL * g, CL * g + CL)
    lc = np.arange(CL)
    gpair = (CL * g + lc) // 2
    invf = THETA ** (-(2.0 * gpair) / C)
    ang = np.arange(T)[None, :] * invf[:, None]
    cosb = np.cos(ang).astype(np.float32)
    sgn = np.where(lc % 2 == 0, -1.0, 1.0)
    sinb = (np.sin(ang) * sgn[:, None]).astype(np.float32)
    # triangular keep-mask (q >= p) duplicated for the two packed heads
    p = np.arange(128)[:, None]
    q = np.arange(128)[None, :]
    tri = (q >= p).astype(np.float32)
    m = np.concatenate([tri, tri], axis=1)
    def pack(a, nk):
        """[nk*128, F] -> [128, nk*F] (k-tiles side by side)."""
        f = a.shape[1]
        return np.ascontiguousarray(
            a.reshape(nk, 128, f).transpose(1, 0, 2).reshape(128, nk * f)
        )

    cs = np.concatenate(
        [cosb[0:128], cosb[128:256], sinb[0:128], sinb[128:256]], axis=1
    )
    return {
        "xT": pack(np.ascontiguousarray(x[b].T), 4).astype(NPBF),
        "wq": pack(np.ascontiguousarray(Wq[sl, :].T), 4).astype(NPBF),
        "wk": pack(np.ascontiguousarray(Wk[sl, :].T), 4).astype(NPBF),
        "wv": pack(np.ascontiguousarray(Wv[sl, :].T), 4).astype(NPBF),
        "wp": pack(np.ascontiguousarray(Wp[:, sl].T), 2).astype(NPBF),
        "cs": np.ascontiguousarray(cs).astype(NPBF),
        "msk": m.astype(NPBF),
    }


_NC_CACHE = {}


def _get_nc(T):
    if T not in _NC_CACHE:
        _NC_CACHE[T] = build_nc(T)
    return _NC_CACHE[T]


def kernel(x, Wq, Wk, Wv, Wp, bp, _trace=False):
    x = np.asarray(x, dtype=np.float32)
    Wq = np.asarray(Wq, dtype=np.float32)
    Wk = np.asarray(Wk, dtype=np.float32)
    Wv = np.asarray(Wv, dtype=np.float32)
    Wp = np.asarray(Wp, dtype=np.float32)
    bp = np.asarray(bp, dtype=np.float32)
    T = x.shape[1]
    nc = _get_nc(T)
    in_maps = [prep_core_inputs(x, Wq, Wk, Wv, Wp, c, T) for c in range(NCORES)]
    if _trace:
        _ensure_ntff_hook()
    res = run_bass_kernel_spmd(nc, in_maps, list(range(NCORES)), trace=_trace)
    out = np.zeros((B, T, C), np.float32)
    for b in range(B):
        out[b] = res.results[2 * b]["y"].astype(np.float32) + res.results[
            2 * b + 1
        ]["y"].astype(np.float32)
    out += bp[None, None, :]
    if _trace:
        return out, res
    return out
